# revision 2
# baseline (speedup 1.0000x reference)
"""Trainium2 Bass kernel for BaseLinearLayerWithLoRA (moe_routing).

out = x @ W^T + b  +  per-token LoRA:  out[t] += (x[t] @ A[l]^T) @ B[l]^T,  l = idx[t]

Sharding: data-parallel over tokens across 8 NeuronCores (4096 tokens each);
W, bias and the stacked LoRA A/B tables are replicated.

Per-core kernel design (single pass over tokens, everything bf16 on the wire):
  - All operands are host-cast to bf16 (x, W, A, B, bias, mask); bf16 matmul
    streams at the same 1 col/cycle PE rate as f32r but halves HBM traffic
    and halves the W-resident startup stall.  PSUM accumulates fp32; output
    is stored fp32.
  - Base GEMM: lhsT = x^T chunks (host-transposed x) stationary, rhs = W^T
    (host-transposed W) moving, W fully resident in SBUF; x is streamed once
    in 256-token super-blocks with a 4-wide o-sweep into 4 PSUM banks.
  - LoRA shrink runs FIRST in each super-block (all 8 adapters at once,
    A_all stacked [128, 2048]): it needs only A + x, so it gives the PE
    work during the initial W load.  A host-precomputed one-hot mask
    (mask[r,t] = r//16==idx[t]) zeroes foreign adapters per token (DVE
    multiply, bf16).  The expand is a single bf16 matmul per output tile
    contracting all 128 adapter-rank rows, accumulated into the same PSUM
    tile as the base GEMM.
  - Bias (host-replicated to 128 rows, bf16) is added during the PSUM->SBUF
    drain; drains are per-PSUM-bank to release banks ASAP.
"""

import sys

for _p in ("/opt/trn_rl_repo", "/root/.axon_site/_ro/trn_rl_repo"):
    if _p not in sys.path:
        sys.path.insert(0, _p)

import numpy as np
import ml_dtypes

import concourse.bass as bass  # noqa: F401  (registers engines)
import concourse.mybir as mybir
import concourse.tile as tile
from concourse import bacc
from concourse.bass_utils import run_bass_kernel_spmd

N_CORES = 8
T_FULL, D_IN, D_OUT = 32768, 2048, 2048
MAX_LORAS, RANK = 8, 16
T_CORE = T_FULL // N_CORES          # 4096 tokens per core
SB_T = 256                          # super-block tokens
N_SB = T_CORE // SB_T               # 16 super-blocks
N_BLK = SB_T // 128                 # 2 token blocks per super-block
KC = D_IN // 128                    # 16 contraction chunks
N_OT = D_OUT // 512                 # 4 o-tiles (full width resident)

_CACHED = {}


def _build():
    if "nc" in _CACHED:
        return _CACHED["nc"]

    f32 = mybir.dt.float32
    bf16 = mybir.dt.bfloat16

    nc = bacc.Bacc("TRN2", target_bir_lowering=False, debug=False)

    xT = nc.dram_tensor("xT", [D_IN, T_CORE], bf16, kind="ExternalInput")
    wT = nc.dram_tensor("wT", [D_IN, D_OUT], bf16, kind="ExternalInput")
    aT = nc.dram_tensor("aT", [D_IN, 128], bf16, kind="ExternalInput")
    bT = nc.dram_tensor("bT", [128, D_OUT], bf16, kind="ExternalInput")
    maskM = nc.dram_tensor("maskM", [128, T_CORE], bf16, kind="ExternalInput")
    biasR = nc.dram_tensor("biasR", [128, D_OUT], bf16, kind="ExternalInput")
    out = nc.dram_tensor("out", [T_CORE, D_OUT], f32, kind="ExternalOutput")

    xT_v = xT.rearrange("(c p) t -> p c t", p=128)      # [128, 16, T_CORE]
    wT_v = wT.rearrange("(c p) o -> p c o", p=128)      # [128, 16, 2048]
    aT_v = aT.rearrange("(c p) r -> p c r", p=128)      # [128, 16, 128]

    with tile.TileContext(nc) as tc:
        with (
            tc.tile_pool(name="const", bufs=1) as const,
            tc.tile_pool(name="wpool", bufs=1) as wpool,
            tc.tile_pool(name="xpool", bufs=3) as xpool,
            tc.tile_pool(name="spool", bufs=2) as spool,
            tc.tile_pool(name="opool", bufs=4) as opool,
            tc.tile_pool(name="pso", bufs=8, space="PSUM") as pso,
        ):
            at = const.tile([128, KC, 128], bf16)
            bt = const.tile([128, D_OUT], bf16)
            bias_t = const.tile([128, D_OUT], bf16)
            mk = const.tile([128, T_CORE], bf16)
            wt = wpool.tile([128, KC, D_OUT], bf16)
            # A first (shrink is the earliest PE work), then W chunk stream.
            nc.sync.dma_start(at[:], aT_v[:])
            for c in range(KC):
                # W loads ride the second HWDGE ring (ACT) so they don't
                # head-of-line-block the x/mask stream on the SP ring.
                nc.scalar.dma_start(wt[:, c, :], wT_v[:, c, :])
            nc.sync.dma_start(mk[:], maskM[:])
            nc.scalar.dma_start(bt[:], bT[:])
            nc.scalar.dma_start(bias_t[:], biasR[:])
            for s in range(N_SB):
                t0 = s * SB_T
                xt = xpool.tile([128, KC, SB_T], bf16, tag="xt", name="xt")
                nc.sync.dma_start(xt[:], xT_v[:, :, t0:t0 + SB_T])
                # LoRA shrink for the whole super-block (all adapters) comes
                # first: it only needs A + x, so at s=0 it runs while W is
                # still streaming in.
                ps_s = pso.tile([128, 512], f32, tag="ps", name="ps_s")
                for c in range(KC):
                    nc.tensor.matmul(ps_s[:, :SB_T], at[:, c, :], xt[:, c, :],
                                     start=(c == 0), stop=(c == KC - 1))
                s_m = spool.tile([128, SB_T], bf16, tag="s_m", name="s_m")
                nc.vector.tensor_tensor(s_m[:], ps_s[:, :SB_T],
                                        mk[:, t0:t0 + SB_T],
                                        mybir.AluOpType.mult)
                for b in range(N_BLK):
                    tb = b * 128
                    psums = [
                        pso.tile([128, 512], f32, tag="ps", name=f"ps_o{o}")
                        for o in range(N_OT)
                    ]
                    for c in range(KC):
                        for o in range(N_OT):
                            nc.tensor.matmul(
                                psums[o][:],
                                xt[:, c, tb:tb + 128],
                                wt[:, c, o * 512:(o + 1) * 512],
                                start=(c == 0), stop=False)
                    for o in range(N_OT):
                        nc.tensor.matmul(
                            psums[o][:],
                            s_m[:, tb:tb + 128],
                            bt[:, o * 512:(o + 1) * 512],
                            start=False, stop=True)
                    # Per-bank drains release PSUM slots ASAP; two stores of
                    # 512 KB each per block on the SP ring.
                    ot = opool.tile([128, D_OUT], f32, tag="ot", name="ot")
                    for o in range(N_OT):
                        nc.vector.tensor_tensor(
                            ot[:, o * 512:(o + 1) * 512], psums[o][:],
                            bias_t[:, o * 512:(o + 1) * 512],
                            mybir.AluOpType.add)
                        if o == 1:
                            nc.sync.dma_start(
                                out[t0 + tb:t0 + tb + 128, :1024],
                                ot[:, :1024])
                    nc.sync.dma_start(
                        out[t0 + tb:t0 + tb + 128, 1024:], ot[:, 1024:])

    nc.compile()
    _CACHED["nc"] = nc
    return nc


def _prep_inputs(x, base_weight, base_bias, lora_a, lora_b, token_lora_indices):
    bf16 = ml_dtypes.bfloat16
    x = np.asarray(x, dtype=np.float32)
    w = np.asarray(base_weight, dtype=np.float32)
    bias = np.asarray(base_bias, dtype=np.float32)
    la = np.asarray(lora_a, dtype=np.float32)
    lb = np.asarray(lora_b, dtype=np.float32)
    idx = np.asarray(token_lora_indices, dtype=np.int32)

    wT = np.ascontiguousarray(w.T).astype(bf16)                      # [D_IN, D_OUT]
    aT = np.ascontiguousarray(la.reshape(128, D_IN).T).astype(bf16)  # [D_IN, 128]
    bT = np.ascontiguousarray(
        lb[:, 0].transpose(0, 2, 1).reshape(128, D_OUT)).astype(bf16)
    biasR = np.ascontiguousarray(
        np.broadcast_to(bias[None, :], (128, D_OUT))).astype(bf16)   # [128, D_OUT]
    mask = (np.arange(128, dtype=np.int32)[:, None] // RANK
            == idx[None, :]).astype(bf16)                            # [128, T_FULL]

    xT = x.T.astype(bf16)                                            # [D_IN, T]
    in_maps = []
    for c in range(N_CORES):
        sl = slice(c * T_CORE, (c + 1) * T_CORE)
        in_maps.append({
            "xT": np.ascontiguousarray(xT[:, sl]),
            "wT": wT,
            "aT": aT,
            "bT": bT,
            "maskM": np.ascontiguousarray(mask[:, sl]),
            "biasR": biasR,
        })
    return in_maps


def kernel(x, base_weight, base_bias, lora_a, lora_b, token_lora_indices):
    nc = _build()
    in_maps = _prep_inputs(x, base_weight, base_bias, lora_a, lora_b,
                           token_lora_indices)
    res = run_bass_kernel_spmd(nc, in_maps, list(range(N_CORES)))
    return np.concatenate([res.results[c]["out"] for c in range(N_CORES)], axis=0)


# revision 4
# speedup vs baseline: 3.4137x; 3.4137x over previous
"""Trainium2 Bass kernel for BaseLinearLayerWithLoRA (moe_routing).

out = x @ W^T + b  +  per-token LoRA:  out[t] += (x[t] @ A[l]^T) @ B[l]^T,  l = idx[t]

Sharding: data-parallel over tokens across 8 NeuronCores (4096 tokens each);
W, bias and the stacked LoRA A/B tables are replicated.

Per-core kernel design (single pass over tokens, everything bf16 on the wire):
  - All operands are host-cast to bf16 (x, W, A, B, bias, mask); bf16 matmul
    streams at the same 1 col/cycle PE rate as f32r, but halves HBM traffic,
    halves the W-resident startup stall, and (unlike f32r) legalizes into
    separate Ldweights+Matmult pairs that pipeline through the PE's reorder
    window instead of self-loading weights serially.  PSUM accumulates fp32;
    output is stored fp32.
  - Host layouts are chosen so every DMA is >=4KB-contiguous per partition:
    x is pre-tiled to [128p, s, c, t] so a super-block load is one 8KB/
    partition descriptor set, A to [128p, c, r].
  - Base GEMM: lhsT = x^T chunks stationary, rhs = W^T moving, W fully
    resident in SBUF; x streamed once in 512-token super-blocks (4 blocks
    of 128 tokens) with a 4-wide o-sweep into 4 PSUM banks per block.
  - LoRA shrink runs FIRST in each super-block (all 8 adapters at once,
    A_all stacked [128, 2048]): it needs only A + x, so it fills the PE
    during the initial W load.  A host-precomputed one-hot mask
    (mask[r,t] = r//16==idx[t]) zeroes foreign adapters per token (DVE
    multiply, bf16).  The expand is one bf16 matmul per output tile
    contracting all 128 adapter-rank rows, into the same PSUM tile as the
    base GEMM.
  - Bias (host-replicated to 128 rows, bf16) is added during per-bank
    PSUM->SBUF drains; output stores alternate between the two HWDGE rings
    so the final-block stores overlap.
"""

import sys

for _p in ("/opt/trn_rl_repo", "/root/.axon_site/_ro/trn_rl_repo"):
    if _p not in sys.path:
        sys.path.insert(0, _p)

import numpy as np
import ml_dtypes

import concourse.bass as bass  # noqa: F401  (registers engines)
import concourse.mybir as mybir
import concourse.tile as tile
from concourse import bacc
from concourse.bass_utils import run_bass_kernel_spmd

N_CORES = 8
T_FULL, D_IN, D_OUT = 32768, 2048, 2048
MAX_LORAS, RANK = 8, 16
T_CORE = T_FULL // N_CORES          # 4096 tokens per core
SB_T = 512                          # super-block tokens
N_SB = T_CORE // SB_T               # 16 super-blocks
N_BLK = SB_T // 128                 # 2 token blocks per super-block
KC = D_IN // 128                    # 16 contraction chunks
N_OT = D_OUT // 512                 # 4 o-tiles (full width resident)

_CACHED = {}


def _build():
    if "nc" in _CACHED:
        return _CACHED["nc"]

    f32 = mybir.dt.float32
    bf16 = mybir.dt.bfloat16

    nc = bacc.Bacc("TRN2", target_bir_lowering=False, debug=False)

    xT = nc.dram_tensor("xT", [128, N_SB, KC, SB_T], bf16, kind="ExternalInput")
    wT = nc.dram_tensor("wT", [D_IN, D_OUT], bf16, kind="ExternalInput")
    aT = nc.dram_tensor("aT", [128, KC, 128], bf16, kind="ExternalInput")
    bT = nc.dram_tensor("bT", [128, D_OUT], bf16, kind="ExternalInput")
    maskM = nc.dram_tensor("maskM", [128, T_CORE], bf16, kind="ExternalInput")
    biasR = nc.dram_tensor("biasR", [128, D_OUT], bf16, kind="ExternalInput")
    out = nc.dram_tensor("out", [T_CORE, D_OUT], f32, kind="ExternalOutput")

    wT_v = wT.rearrange("(c p) o -> p c o", p=128)      # [128, 16, 2048]

    with tile.TileContext(nc) as tc:
        with (
            tc.tile_pool(name="const", bufs=1) as const,
            tc.tile_pool(name="wpool", bufs=1) as wpool,
            tc.tile_pool(name="xpool", bufs=2) as xpool,
            tc.tile_pool(name="spool", bufs=2) as spool,
            tc.tile_pool(name="opool", bufs=8) as opool,
            tc.tile_pool(name="pso", bufs=8, space="PSUM") as pso,
        ):
            at = const.tile([128, KC, 128], bf16)
            bt = const.tile([128, D_OUT], bf16)
            bias_t = const.tile([128, D_OUT], bf16)
            mk = const.tile([128, T_CORE], bf16)
            wt = wpool.tile([128, KC, D_OUT], bf16)
            # Startup order on the SP ring: A table, then the first x
            # super-block (shrink is the earliest PE work), then the mask.
            # The W chunk stream rides the second HWDGE ring (ACT) so it
            # doesn't head-of-line-block the x stream.
            nc.scalar.dma_start(at[:], aT[:])
            xt0 = xpool.tile([128, KC, SB_T], bf16, tag="xt", name="xt")
            for q in range(4):
                nc.sync.dma_start(xt0[:, q * (KC // 4):(q + 1) * (KC // 4), :],
                                  xT[:, 0, q * (KC // 4):(q + 1) * (KC // 4), :])
            for c in range(KC):
                weng = nc.scalar if c % 2 else nc.sync
                weng.dma_start(wt[:, c, :], wT_v[:, c, :])
            for s in range(N_SB):
                t0 = s * SB_T
                if s == 0:
                    xt = xt0
                else:
                    xt = xpool.tile([128, KC, SB_T], bf16, tag="xt", name="xt")
                    nc.sync.dma_start(xt[:], xT[:, s, :, :])
                if s == 0:
                    nc.sync.dma_start(mk[:], maskM[:])
                    nc.sync.dma_start(bt[:], bT[:])
                    nc.sync.dma_start(bias_t[:], biasR[:])
                # LoRA shrink for the whole super-block (all adapters) comes
                # first: it only needs A + x, so at s=0 it runs while W is
                # still streaming in.
                ps_s = pso.tile([128, 512], f32, tag="ps", name="ps_s")
                for c in range(KC):
                    nc.tensor.matmul(ps_s[:, :SB_T], at[:, c, :], xt[:, c, :],
                                     start=(c == 0), stop=(c == KC - 1))
                s_m = spool.tile([128, SB_T], bf16, tag="s_m", name="s_m")
                nc.vector.tensor_tensor(s_m[:], ps_s[:, :SB_T],
                                        mk[:, t0:t0 + SB_T],
                                        mybir.AluOpType.mult)
                for b in range(N_BLK):
                    tb = b * 128
                    psums = [
                        pso.tile([128, 512], f32, tag="ps", name=f"ps_o{o}")
                        for o in range(N_OT)
                    ]
                    for c in range(KC):
                        for o in range(N_OT):
                            nc.tensor.matmul(
                                psums[o][:],
                                xt[:, c, tb:tb + 128],
                                wt[:, c, o * 512:(o + 1) * 512],
                                start=(c == 0), stop=False)
                    for o in range(N_OT):
                        nc.tensor.matmul(
                            psums[o][:],
                            s_m[:, tb:tb + 128],
                            bt[:, o * 512:(o + 1) * 512],
                            start=False, stop=True)
                    # Per-bank drains release PSUM slots ASAP; the four
                    # 512KB stores alternate across the two HWDGE rings so
                    # the tail overlaps.
                    for o in range(N_OT):
                        ot = opool.tile([128, 512], f32, tag="ot", name="ot")
                        nc.vector.tensor_tensor(
                            ot[:], psums[o][:],
                            bias_t[:, o * 512:(o + 1) * 512],
                            mybir.AluOpType.add)
                        eng = nc.sync if o % 2 == 0 else nc.scalar
                        eng.dma_start(
                            out[t0 + tb:t0 + tb + 128, o * 512:(o + 1) * 512],
                            ot[:])

    nc.compile()
    _CACHED["nc"] = nc
    return nc


def _prep_inputs(x, base_weight, base_bias, lora_a, lora_b, token_lora_indices):
    bf16 = ml_dtypes.bfloat16
    x = np.asarray(x, dtype=np.float32)
    w = np.asarray(base_weight, dtype=np.float32)
    bias = np.asarray(base_bias, dtype=np.float32)
    la = np.asarray(lora_a, dtype=np.float32)
    lb = np.asarray(lora_b, dtype=np.float32)
    idx = np.asarray(token_lora_indices, dtype=np.int32)

    wT = np.ascontiguousarray(w.T).astype(bf16)                      # [D_IN, D_OUT]
    # A_all^T [D_IN, 128] -> [p, c, r] so the SBUF tile load is contiguous
    # 4KB per partition.
    aT = np.ascontiguousarray(
        la.reshape(128, D_IN).T.reshape(KC, 128, 128).transpose(1, 0, 2)
    ).astype(bf16)                                                   # [128, 16, 128]
    bT = np.ascontiguousarray(
        lb[:, 0].transpose(0, 2, 1).reshape(128, D_OUT)).astype(bf16)
    biasR = np.ascontiguousarray(
        np.broadcast_to(bias[None, :], (128, D_OUT))).astype(bf16)   # [128, D_OUT]
    mask = (np.arange(128, dtype=np.int32)[:, None] // RANK
            == idx[None, :]).astype(bf16)                            # [128, T_FULL]

    in_maps = []
    for c in range(N_CORES):
        sl = slice(c * T_CORE, (c + 1) * T_CORE)
        # x_core [T_CORE, D_IN] -> [p, s, c, t]: per-partition 8KB-contiguous
        # super-block loads.
        xc = x[sl].reshape(N_SB, SB_T, KC, 128).transpose(3, 0, 2, 1)
        in_maps.append({
            "xT": np.ascontiguousarray(xc).astype(bf16),
            "wT": wT,
            "aT": aT,
            "bT": bT,
            "maskM": np.ascontiguousarray(mask[:, sl]),
            "biasR": biasR,
        })
    return in_maps


def kernel(x, base_weight, base_bias, lora_a, lora_b, token_lora_indices):
    nc = _build()
    in_maps = _prep_inputs(x, base_weight, base_bias, lora_a, lora_b,
                           token_lora_indices)
    res = run_bass_kernel_spmd(nc, in_maps, list(range(N_CORES)))
    return np.concatenate([res.results[c]["out"] for c in range(N_CORES)], axis=0)
